# revision 13
# baseline (speedup 1.0000x reference)
"""2-layer GraphSAGE (mean aggr + BN(eval) + ReLU) on Trainium2, 8-core SPMD.

Strategy (graph/data parallel, dst-node sharding, host-mediated all-to-all):
  - Host: relabel nodes by in-degree (descending), deal 128-node chunks
    round-robin to the 8 cores so chunk ci holds same-degree nodes on every
    core (shared per-chunk pad depth K[ci], SPMD). The host performs the
    all-to-all exchange of source features: for each core it stages the
    edge-gathered source-feature slabs expT[ch, slot] (bf16, channel-major,
    slot = (chunk, k, dst-lane), zero-padded to K[ci] in-edges per node).
  - Device layer (identical structure for both layers):
      expT streams into SBUF in 8 big pipelined section DMAs (it stays
      resident: 154KB/partition). Per chunk:
        ps_A = sum_k slab_k^T @ Wproj      (= agg^T @ Wproj, K matmuls
                                            accumulated in PSUM)
        ps_B = own^T @ Wself + ones^T @ brow   (bias via K=1 matmul)
        out  = ps_A * invdeg[dst] + ps_B   (DVE scalar_tensor_tensor,
                                            invdeg fp32 per-partition)
      (+ ReLU for layer 1). Outputs collect in SBUF lane-major and are
      written once at the end ([128, CPC*chout], host unshuffles).
  - Between launches the host assembles h, re-runs the same index map to
    stage layer 2's slabs (all-to-all of h), and unpermutes the final out.
"""

import numpy as np

import concourse.bacc as bacc
import concourse.mybir as mybir
import concourse.tile as tile
from concourse.bass_utils import run_bass_kernel_spmd

F32 = mybir.dt.float32
BF16 = mybir.dt.bfloat16
OP = mybir.AluOpType
BF16_NP = mybir.dt.np(mybir.dt.bfloat16)

N_CORES = 8
P = 128

N_NODES = 50000
NP_PAD = 50176            # 392 chunks of 128
E = 600000
C_IN, C_HID, C_OUT = 128, 128, 64
CPC = NP_PAD // P // N_CORES   # 49 chunks per core
NPC = CPC * P                  # 6272 nodes per core
BN_EPS = 1e-5
NSEC = 8                       # expT section loads


def _preprocess(edge_index):
    """Degree-sort relabeling + slot map for the edge-gathered slabs."""
    src = np.asarray(edge_index[0]).astype(np.int64)
    dst = np.asarray(edge_index[1]).astype(np.int64)
    ne = src.shape[0]
    deg = np.bincount(dst, minlength=NP_PAD).astype(np.int64)

    nodeorder = np.argsort(-deg, kind="stable")        # rank -> node
    rank = np.empty(NP_PAD, np.int64)
    rank[nodeorder] = np.arange(NP_PAD)

    gdeg = deg[nodeorder].reshape(NP_PAD // P, P)
    K = np.maximum(gdeg.reshape(CPC, N_CORES, P).max(axis=(1, 2)), 1)
    colstart = np.zeros(CPC, np.int64)
    colstart[1:] = np.cumsum(K)[:-1]
    S_total = int(K.sum())

    key = rank[dst]
    order = np.argsort(key, kind="stable")
    r_s = key[order]
    src_s = src[order].astype(np.int32)
    starts = np.searchsorted(r_s, r_s, side="left")
    k_in = np.arange(ne) - starts
    g = r_s // P
    core = g % N_CORES
    ci = g // N_CORES
    p = r_s % P
    J = (colstart[ci] + k_in) * P + p
    slot_src = []
    for c in range(N_CORES):
        m = core == c
        a = np.full(S_total * P, -1, np.int32)
        a[J[m]] = src_s[m]
        slot_src.append(a)

    node_of = []
    ivd_t = (1.0 / np.maximum(deg, 1.0)).astype(np.float32)
    slot_scale = []
    for c in range(N_CORES):
        idx = (np.arange(CPC)[:, None] * N_CORES + c) * P + np.arange(P)[None, :]
        nodes = nodeorder[idx]                         # [CPC, P]
        node_of.append(nodes.reshape(-1).astype(np.int32))
        iv = ivd_t[nodes]                              # [CPC, P]
        sc = np.concatenate(
            [np.tile(iv[ci], int(K[ci])) for ci in range(CPC)])
        slot_scale.append(sc.astype(np.float32))       # [S_total*P]
    return K, slot_src, node_of, slot_scale


def _mk_nc():
    return bacc.Bacc(
        "TRN2",
        target_bir_lowering=False,
        debug=False,
        enable_asserts=False,
        num_devices=N_CORES,
    )


def build_layer(K, chout, relu, out_bf16):
    """One GraphSAGE layer. expT slabs (pre-scaled by invdeg) + own + W -> out."""
    S_total = int(K.sum())
    csum = np.zeros(CPC + 1, np.int64)
    csum[1:] = np.cumsum(K)
    # section boundaries (chunk indices): progressive sizes — small first so
    # compute starts early, growing as the pipeline fills
    fracs = np.cumsum([0, 2, 3, 5, 8, 12, 17, 24, 29])
    fracs = fracs / fracs[-1]
    bounds = [0]
    for s in range(1, NSEC):
        b = int(np.searchsorted(csum, S_total * fracs[s]))
        bounds.append(max(b, bounds[-1]))
    bounds.append(CPC)

    nc = _mk_nc()
    d_exp = nc.dram_tensor("expT", (P, S_total * P), BF16, kind="ExternalInput")
    d_own = nc.dram_tensor("ownT", (P, NPC), BF16, kind="ExternalInput")
    d_wa = nc.dram_tensor("wa", (C_IN, chout), BF16, kind="ExternalInput")
    d_wb = nc.dram_tensor("wb", (C_IN, chout), BF16, kind="ExternalInput")
    d_brow = nc.dram_tensor("brow", (1, chout), BF16, kind="ExternalInput")
    d_ones = nc.dram_tensor("ones", (1, P), BF16, kind="ExternalInput")
    out_dt = BF16 if out_bf16 else F32
    d_out = nc.dram_tensor("out", (P, CPC * chout), out_dt, kind="ExternalOutput")
    AF = mybir.ActivationFunctionType

    with tile.TileContext(nc) as tc:
        with (
            tc.tile_pool(name="const", bufs=1) as cp,
            tc.tile_pool(name="psA", bufs=6, space="PSUM") as pA,
        ):
            def cload(name, d, shape, dt=BF16):
                # scalar-engine HWDGE ring: parallel to the big section loads
                t = cp.tile(shape, dt, tag=name)
                nc.scalar.dma_start(t[:], d.ap()[:, :])
                return t

            t_exp = cp.tile([P, S_total * P], BF16, tag="exp")
            for s in range(NSEC):
                a = int(csum[bounds[s]]) * P
                b = int(csum[bounds[s + 1]]) * P
                if b > a:
                    nc.sync.dma_start(t_exp[:, a:b], d_exp.ap()[:, a:b])
            t_wa = cload("wa", d_wa, [C_IN, chout])
            t_own = cload("own", d_own, [P, NPC])
            t_wb = cload("wb", d_wb, [C_IN, chout])
            t_brow = cload("brow", d_brow, [1, chout])
            t_ones = cload("ones", d_ones, [1, P])
            t_hall = cp.tile([P, CPC * chout], out_dt, tag="hall")

            for ci in range(CPC):
                k = int(K[ci])
                c0 = int(csum[ci])
                ps = pA.tile([P, chout], F32)
                for kk in range(k):
                    nc.tensor.matmul(
                        out=ps[:],
                        lhsT=t_exp[:, (c0 + kk) * P:(c0 + kk + 1) * P],
                        rhs=t_wa[:],
                        start=(kk == 0),
                        stop=False,
                    )
                nc.tensor.matmul(out=ps[:],
                                 lhsT=t_own[:, ci * P:(ci + 1) * P],
                                 rhs=t_wb[:], start=False, stop=False)
                nc.tensor.matmul(out=ps[:], lhsT=t_ones[:], rhs=t_brow[:],
                                 start=False, stop=True)
                dst_sl = t_hall[:, ci * chout:(ci + 1) * chout]
                nc.scalar.activation(out=dst_sl, in_=ps[:],
                                     func=AF.Relu if relu else AF.Identity)
                # flush finished output stripes while compute continues
                if ci in (15, 31, 43, CPC - 1):
                    prev = {15: 0, 31: 16, 43: 32, CPC - 1: 44}[ci]
                    nc.sync.dma_start(
                        d_out.ap()[:, prev * chout:(ci + 1) * chout],
                        t_hall[:, prev * chout:(ci + 1) * chout])

    nc.compile()
    return nc


_cache = {}


def _get_programs(K):
    key = tuple(int(x) for x in K)
    if key not in _cache:
        _cache[key] = (
            build_layer(K, C_HID, relu=True, out_bf16=True),
            build_layer(K, C_OUT, relu=False, out_bf16=False),
        )
    return _cache[key]


def _expand(tabT_ext, slot_idx, scale):
    """tabT_ext f32 [128, NP_PAD+1] (last col zero), slot_idx int32 (-1 pad),
    scale f32 per slot column; single rounding to bf16."""
    idx = np.where(slot_idx < 0, NP_PAD, slot_idx)
    e = tabT_ext[:, idx] * scale[None, :]
    return np.ascontiguousarray(e.astype(BF16_NP))


def _unshuffle(part, chout):
    """[P, CPC*chout] lane-major -> [NPC, chout] row-major."""
    return np.ascontiguousarray(
        part.reshape(P, CPC, chout).transpose(1, 0, 2).reshape(NPC, chout))


def kernel(x, edge_index, W1_l, W1_r, b1, bn_gamma, bn_beta, bn_mean, bn_var,
           W2_l, W2_r, b2, _results=None):
    K, slot_src, node_of, slot_scale = _preprocess(edge_index)
    nc1, nc2 = _get_programs(K)

    # BN folding (float64 for accuracy): h = gamma*(z - mean)/sqrt(var+eps)+beta
    s = (np.asarray(bn_gamma, np.float64)
         / np.sqrt(np.asarray(bn_var, np.float64) + BN_EPS))
    w1l_f = (np.asarray(W1_l, np.float64) * s[None, :]).astype(BF16_NP)
    w1r_f = (np.asarray(W1_r, np.float64) * s[None, :]).astype(BF16_NP)
    c1 = ((np.asarray(b1, np.float64) - np.asarray(bn_mean, np.float64)) * s
          + np.asarray(bn_beta, np.float64)).astype(BF16_NP).reshape(1, C_HID)
    w2l = np.asarray(W2_l, np.float32).astype(BF16_NP)
    w2r = np.asarray(W2_r, np.float32).astype(BF16_NP)
    b2r = np.asarray(b2, np.float32).astype(BF16_NP).reshape(1, C_OUT)
    ones = np.ones((1, P), np.float32).astype(BF16_NP)

    x_pad = np.zeros((NP_PAD + 1, C_IN), np.float32)
    x_pad[:N_NODES] = np.asarray(x, np.float32)
    # round features once to bf16, then expand/scale from the f32 view of that
    xT_bf = np.ascontiguousarray(x_pad.astype(BF16_NP).T)   # [128, NP+1]
    xT_f = xT_bf.astype(np.float32)

    maps1 = []
    for c in range(N_CORES):
        maps1.append(dict(
            expT=_expand(xT_f, slot_src[c], slot_scale[c]),
            ownT=np.ascontiguousarray(xT_bf[:, node_of[c]]),
            wa=w1l_f, wb=w1r_f, brow=c1, ones=ones,
        ))
    r1 = run_bass_kernel_spmd(nc1, maps1, list(range(N_CORES)))

    # assemble h (original node ids), then all-to-all for layer 2
    hT_bf = np.zeros((C_HID, NP_PAD + 1), BF16_NP)
    for c in range(N_CORES):
        h_part = _unshuffle(np.asarray(r1.results[c]["out"]), C_HID)
        hT_bf[:, node_of[c]] = h_part.T
    hT_bf[:, NP_PAD] = 0
    hT_f = hT_bf.astype(np.float32)

    maps2 = []
    for c in range(N_CORES):
        maps2.append(dict(
            expT=_expand(hT_f, slot_src[c], slot_scale[c]),
            ownT=np.ascontiguousarray(hT_bf[:, node_of[c]]),
            wa=w2l, wb=w2r, brow=b2r, ones=ones,
        ))
    r2 = run_bass_kernel_spmd(nc2, maps2, list(range(N_CORES)))

    out = np.zeros((NP_PAD, C_OUT), np.float32)
    for c in range(N_CORES):
        out[node_of[c]] = _unshuffle(np.asarray(r2.results[c]["out"]), C_OUT)
    if _results is not None:
        _results.extend([r1, r2])
    return np.ascontiguousarray(out[:N_NODES])


# revision 16
# speedup vs baseline: 1.1114x; 1.1114x over previous
"""2-layer GraphSAGE (mean aggr + BN(eval) + ReLU) on Trainium2, 8-core SPMD.

Strategy (graph/data parallel, dst-node sharding, host-mediated all-to-all):
  - Host: relabel nodes by in-degree (descending), deal 128-node chunks
    round-robin to the 8 cores so chunk ci holds same-degree nodes on every
    core (shared per-chunk pad depth K[ci], SPMD). The host performs the
    all-to-all exchange of source features: for each core it stages the
    edge-gathered source-feature slabs expT[ch, slot] (bf16, channel-major,
    slot = (chunk, k, dst-lane), zero-padded to K[ci] in-edges per node).
  - Device layer (identical structure for both layers):
      expT streams into SBUF in 8 big pipelined section DMAs (it stays
      resident: 154KB/partition). Per chunk:
        ps_A = sum_k slab_k^T @ Wproj      (= agg^T @ Wproj, K matmuls
                                            accumulated in PSUM)
        ps_B = own^T @ Wself + ones^T @ brow   (bias via K=1 matmul)
        out  = ps_A * invdeg[dst] + ps_B   (DVE scalar_tensor_tensor,
                                            invdeg fp32 per-partition)
      (+ ReLU for layer 1). Outputs collect in SBUF lane-major and are
      written once at the end ([128, CPC*chout], host unshuffles).
  - Between launches the host assembles h, re-runs the same index map to
    stage layer 2's slabs (all-to-all of h), and unpermutes the final out.
"""

import numpy as np

import concourse.bacc as bacc
import concourse.mybir as mybir
import concourse.tile as tile
from concourse.bass_utils import run_bass_kernel_spmd

F32 = mybir.dt.float32
BF16 = mybir.dt.bfloat16
OP = mybir.AluOpType
BF16_NP = mybir.dt.np(mybir.dt.bfloat16)

N_CORES = 8
P = 128

N_NODES = 50000
NP_PAD = 50176            # 392 chunks of 128
E = 600000
C_IN, C_HID, C_OUT = 128, 128, 64
CPC = NP_PAD // P // N_CORES   # 49 chunks per core
NPC = CPC * P                  # 6272 nodes per core
BN_EPS = 1e-5
NSEC = 16                      # expT section loads


def _preprocess(edge_index):
    """Degree-sort relabeling + slot map for the edge-gathered slabs."""
    src = np.asarray(edge_index[0]).astype(np.int64)
    dst = np.asarray(edge_index[1]).astype(np.int64)
    ne = src.shape[0]
    deg = np.bincount(dst, minlength=NP_PAD).astype(np.int64)

    nodeorder = np.argsort(-deg, kind="stable")        # rank -> node
    rank = np.empty(NP_PAD, np.int64)
    rank[nodeorder] = np.arange(NP_PAD)

    gdeg = deg[nodeorder].reshape(NP_PAD // P, P)
    K = np.maximum(gdeg.reshape(CPC, N_CORES, P).max(axis=(1, 2)), 1)
    colstart = np.zeros(CPC, np.int64)
    colstart[1:] = np.cumsum(K)[:-1]
    S_total = int(K.sum())

    key = rank[dst]
    order = np.argsort(key, kind="stable")
    r_s = key[order]
    src_s = src[order].astype(np.int32)
    starts = np.searchsorted(r_s, r_s, side="left")
    k_in = np.arange(ne) - starts
    g = r_s // P
    core = g % N_CORES
    ci = g // N_CORES
    p = r_s % P
    J = (colstart[ci] + k_in) * P + p
    slot_src = []
    for c in range(N_CORES):
        m = core == c
        a = np.full(S_total * P, -1, np.int32)
        a[J[m]] = src_s[m]
        slot_src.append(a)

    node_of = []
    ivd_t = (1.0 / np.maximum(deg, 1.0)).astype(np.float32)
    slot_scale = []
    for c in range(N_CORES):
        idx = (np.arange(CPC)[:, None] * N_CORES + c) * P + np.arange(P)[None, :]
        nodes = nodeorder[idx]                         # [CPC, P]
        node_of.append(nodes.reshape(-1).astype(np.int32))
        iv = ivd_t[nodes]                              # [CPC, P]
        sc = np.concatenate(
            [np.tile(iv[ci], int(K[ci])) for ci in range(CPC)])
        slot_scale.append(sc.astype(np.float32))       # [S_total*P]
    return K, slot_src, node_of, slot_scale


def _mk_nc():
    return bacc.Bacc(
        "TRN2",
        target_bir_lowering=False,
        debug=False,
        enable_asserts=False,
        num_devices=N_CORES,
    )


def build_layer(K, chout, relu, out_bf16):
    """One GraphSAGE layer. expT slabs (pre-scaled by invdeg) + own + W -> out."""
    S_total = int(K.sum())
    csum = np.zeros(CPC + 1, np.int64)
    csum[1:] = np.cumsum(K)
    # section boundaries (chunk indices): progressive sizes — small first so
    # compute starts early, growing as the pipeline fills
    fracs = np.cumsum([0, 1.5, 1.5, 2, 3, 5, 6, 7, 8, 8.5,
                       9, 9, 9, 9, 9, 9, 3.5])
    fracs = fracs / fracs[-1]
    bounds = [0]
    for s in range(1, NSEC):
        b = int(np.searchsorted(csum, S_total * fracs[s]))
        bounds.append(max(b, bounds[-1]))
    bounds.append(CPC)

    nc = _mk_nc()
    d_exp = nc.dram_tensor("expT", (P, S_total * P), BF16, kind="ExternalInput")
    d_own = nc.dram_tensor("ownT", (P, NPC), BF16, kind="ExternalInput")
    d_wa = nc.dram_tensor("wa", (C_IN, chout), BF16, kind="ExternalInput")
    d_wb = nc.dram_tensor("wb", (C_IN, chout), BF16, kind="ExternalInput")
    d_brow = nc.dram_tensor("brow", (1, chout), BF16, kind="ExternalInput")
    d_ones = nc.dram_tensor("ones", (1, P), BF16, kind="ExternalInput")
    out_dt = BF16 if out_bf16 else F32
    d_out = nc.dram_tensor("out", (P, CPC * chout), out_dt, kind="ExternalOutput")
    AF = mybir.ActivationFunctionType

    with tile.TileContext(nc) as tc:
        with (
            tc.tile_pool(name="const", bufs=1) as cp,
            tc.tile_pool(name="psA", bufs=4, space="PSUM") as pA,
            tc.tile_pool(name="psW", bufs=1, space="PSUM") as pW,
        ):
            def cload(name, d, shape, dt=BF16):
                # scalar-engine HWDGE ring: parallel to the big section loads
                t = cp.tile(shape, dt, tag=name)
                nc.scalar.dma_start(t[:], d.ap()[:, :])
                return t

            t_exp = cp.tile([P, S_total * P], BF16, tag="exp")
            for s in range(NSEC):
                a = int(csum[bounds[s]]) * P
                b = int(csum[bounds[s + 1]]) * P
                if b > a:
                    nc.sync.dma_start(t_exp[:, a:b], d_exp.ap()[:, a:b])
            t_wa = cload("wa", d_wa, [C_IN, chout])
            t_own = cload("own", d_own, [P, NPC])
            t_wb = cload("wb", d_wb, [C_IN, chout])
            t_brow = cload("brow", d_brow, [1, chout])
            t_ones = cload("ones", d_ones, [1, P])
            t_hall = cp.tile([P, CPC * chout], out_dt, tag="hall")

            # HAM warmup: keep the PE busy while the first sections stream in
            ps_w = pW.tile([chout, chout], F32)
            for w in range(40):
                nc.tensor.matmul(out=ps_w[:], lhsT=t_wa[:], rhs=t_wa[:],
                                 start=(w == 0), stop=(w == 39))

            for ci in range(CPC):
                k = int(K[ci])
                c0 = int(csum[ci])
                ps = pA.tile([P, chout], F32)
                for kk in range(k):
                    nc.tensor.matmul(
                        out=ps[:],
                        lhsT=t_exp[:, (c0 + kk) * P:(c0 + kk + 1) * P],
                        rhs=t_wa[:],
                        start=(kk == 0),
                        stop=False,
                    )
                nc.tensor.matmul(out=ps[:],
                                 lhsT=t_own[:, ci * P:(ci + 1) * P],
                                 rhs=t_wb[:], start=False, stop=False)
                nc.tensor.matmul(out=ps[:], lhsT=t_ones[:], rhs=t_brow[:],
                                 start=False, stop=True)
                dst_sl = t_hall[:, ci * chout:(ci + 1) * chout]
                nc.scalar.activation(out=dst_sl, in_=ps[:],
                                     func=AF.Relu if relu else AF.Identity)
                # flush finished output stripes while compute continues
                if ci in (15, 31, 43, CPC - 1):
                    prev = {15: 0, 31: 16, 43: 32, CPC - 1: 44}[ci]
                    nc.scalar.dma_start(
                        d_out.ap()[:, prev * chout:(ci + 1) * chout],
                        t_hall[:, prev * chout:(ci + 1) * chout])

    nc.compile()
    return nc


_cache = {}


def _get_programs(K):
    key = tuple(int(x) for x in K)
    if key not in _cache:
        _cache[key] = (
            build_layer(K, C_HID, relu=True, out_bf16=True),
            build_layer(K, C_OUT, relu=False, out_bf16=False),
        )
    return _cache[key]


def _expand(tabT_ext, slot_idx, scale):
    """tabT_ext f32 [128, NP_PAD+1] (last col zero), slot_idx int32 (-1 pad),
    scale f32 per slot column; single rounding to bf16."""
    idx = np.where(slot_idx < 0, NP_PAD, slot_idx)
    e = tabT_ext[:, idx] * scale[None, :]
    return np.ascontiguousarray(e.astype(BF16_NP))


def _unshuffle(part, chout):
    """[P, CPC*chout] lane-major -> [NPC, chout] row-major."""
    return np.ascontiguousarray(
        part.reshape(P, CPC, chout).transpose(1, 0, 2).reshape(NPC, chout))


def kernel(x, edge_index, W1_l, W1_r, b1, bn_gamma, bn_beta, bn_mean, bn_var,
           W2_l, W2_r, b2, _results=None):
    K, slot_src, node_of, slot_scale = _preprocess(edge_index)
    nc1, nc2 = _get_programs(K)

    # BN folding (float64 for accuracy): h = gamma*(z - mean)/sqrt(var+eps)+beta
    s = (np.asarray(bn_gamma, np.float64)
         / np.sqrt(np.asarray(bn_var, np.float64) + BN_EPS))
    w1l_f = (np.asarray(W1_l, np.float64) * s[None, :]).astype(BF16_NP)
    w1r_f = (np.asarray(W1_r, np.float64) * s[None, :]).astype(BF16_NP)
    c1 = ((np.asarray(b1, np.float64) - np.asarray(bn_mean, np.float64)) * s
          + np.asarray(bn_beta, np.float64)).astype(BF16_NP).reshape(1, C_HID)
    w2l = np.asarray(W2_l, np.float32).astype(BF16_NP)
    w2r = np.asarray(W2_r, np.float32).astype(BF16_NP)
    b2r = np.asarray(b2, np.float32).astype(BF16_NP).reshape(1, C_OUT)
    ones = np.ones((1, P), np.float32).astype(BF16_NP)

    x_pad = np.zeros((NP_PAD + 1, C_IN), np.float32)
    x_pad[:N_NODES] = np.asarray(x, np.float32)
    # round features once to bf16, then expand/scale from the f32 view of that
    xT_bf = np.ascontiguousarray(x_pad.astype(BF16_NP).T)   # [128, NP+1]
    xT_f = xT_bf.astype(np.float32)

    maps1 = []
    for c in range(N_CORES):
        maps1.append(dict(
            expT=_expand(xT_f, slot_src[c], slot_scale[c]),
            ownT=np.ascontiguousarray(xT_bf[:, node_of[c]]),
            wa=w1l_f, wb=w1r_f, brow=c1, ones=ones,
        ))
    r1 = run_bass_kernel_spmd(nc1, maps1, list(range(N_CORES)))

    # assemble h (original node ids), then all-to-all for layer 2
    hT_bf = np.zeros((C_HID, NP_PAD + 1), BF16_NP)
    for c in range(N_CORES):
        h_part = _unshuffle(np.asarray(r1.results[c]["out"]), C_HID)
        hT_bf[:, node_of[c]] = h_part.T
    hT_bf[:, NP_PAD] = 0
    hT_f = hT_bf.astype(np.float32)

    maps2 = []
    for c in range(N_CORES):
        maps2.append(dict(
            expT=_expand(hT_f, slot_src[c], slot_scale[c]),
            ownT=np.ascontiguousarray(hT_bf[:, node_of[c]]),
            wa=w2l, wb=w2r, brow=b2r, ones=ones,
        ))
    r2 = run_bass_kernel_spmd(nc2, maps2, list(range(N_CORES)))

    out = np.zeros((NP_PAD, C_OUT), np.float32)
    for c in range(N_CORES):
        out[node_of[c]] = _unshuffle(np.asarray(r2.results[c]["out"]), C_OUT)
    if _results is not None:
        _results.extend([r1, r2])
    return np.ascontiguousarray(out[:N_NODES])
